# revision 14
# baseline (speedup 1.0000x reference)
"""Trainium2 Bass kernel for nn_MOAB_46273977647401.

Network (reference):
  x1 (256,256), x3 (256,) -> 4 outer sigmoid maps (256,257,257)
  -> 1x1 conv combine (4ch) + eval BN + leaky(0.1) -> (256, 66049)
  -> FC (66049 -> 512) + relu -> FC (512 -> 4)

Sharding: 8-way split of the FC contraction dim K = 257*257 by the j
(column) index: core c owns j in [1+32c, 33+32c); the j=0 column and the
i=256 strip's j=0 element are computed identically on every core against
1/8-scaled weights (outputs are partial sums, reduced on host).

Per core, z maps live in [i-partitions, (j,b)-free] layout. The a row
values (and their host-precomputed reciprocals) are DMA-broadcast from
DRAM to all 128 partitions as bf16; ScalarE computes the 4 sigmoid maps
with per-partition bias/scale; DVE+Pool do the conv+BN-folded combine +
leaky; PE accumulates out[h, b] PSUM tiles with the fc_w slab as the
stationary (lhsT) operand. Dependency-free filler matmuls into a scratch
PSUM bank keep the PE p-state ramped between real accumulation bursts.

Host: sums the 8 partial (512, 256) outputs, applies fc bias + relu and
the tiny 512->4 output layer.
"""

import numpy as np

import concourse.bass as bass
import concourse.tile as tile
from concourse import bacc, mybir
from concourse.bass_utils import run_bass_kernel_spmd

F32 = mybir.dt.float32
BF16 = mybir.dt.bfloat16
AL = mybir.AluOpType
SIG = mybir.ActivationFunctionType.Sigmoid

B, N, H, C = 256, 256, 512, 4
NP = 257                  # N+1
NCORE = 8
JPC = 32                  # j columns per core (j in [1+32c, 33+32c))
JC = 8                    # j values per chunk
CH = JC * B               # 2048 free elems per chunk
NCHUNK = JPC // JC        # 4
NHT = H // 128            # 4 output h tiles
EPS = 1e-10
BN_EPS = 1e-5
LEAKY = 0.1
PRE_WARM = 26             # PE keep-warm matmuls before the first real burst
ND_HALF = 10              # keep-warm matmuls after each half-burst
ND_TAIL = 19              # keep-warm matmuls before the final iteration


def build_program():
    nc = bacc.Bacc("TRN2", target_bir_lowering=False, debug=False, num_devices=8)

    d_arows = nc.dram_tensor("arows", [1, NCHUNK * CH], BF16,
                             kind="ExternalInput").ap()
    d_rrows = nc.dram_tensor("rrows", [1, NCHUNK * CH], BF16,
                             kind="ExternalInput").ap()
    d_bcols = nc.dram_tensor("bcols", [2 * 128, 2], F32, kind="ExternalInput").ap()
    d_svcv = nc.dram_tensor("svcv", [128, 9], F32, kind="ExternalInput").ap()
    d_wmain = nc.dram_tensor("wmain", [2 * NCHUNK * 128, JC * H], BF16,
                             kind="ExternalInput").ap()
    d_wcol0 = nc.dram_tensor("wcol0", [128, 2 * H], BF16, kind="ExternalInput").ap()
    d_wstrip = nc.dram_tensor("wstrip", [JPC + 1, H], BF16, kind="ExternalInput").ap()
    d_stripa = nc.dram_tensor("stripa", [JPC + 1, 3 * B], BF16,
                              kind="ExternalInput").ap()
    d_out = nc.dram_tensor("out", [128, NHT * B], F32, kind="ExternalOutput").ap()

    JS = JPC + 1  # strip partitions

    with tile.TileContext(nc) as tc:
        with (
            tc.tile_pool(name="const", bufs=1) as cpool,
            tc.tile_pool(name="sp", bufs=1) as spool,
            tc.tile_pool(name="w", bufs=3) as wpool,
            tc.tile_pool(name="ar", bufs=2) as arpool,
            tc.tile_pool(name="z", bufs=2) as zpool,
            tc.tile_pool(name="comb", bufs=2) as combpool,
            tc.tile_pool(name="ylp", bufs=3) as ylpool,
            tc.tile_pool(name="fin", bufs=1) as finpool,
            tc.tile_pool(name="psO", bufs=1, space="PSUM") as psO,
            tc.tile_pool(name="psD", bufs=1, space="PSUM") as psD,
        ):
            # ------- PE keep-warm scratch (no external deps) ---------------
            dum = cpool.tile([128, 512], BF16, tag="dum")
            nc.gpsimd.memset(dum[:, :], 0.0)
            pd = psD.tile([128, 512], F32, tag="pd")

            def warm(n):
                for _ in range(n):
                    nc.tensor.matmul(pd[:, :], dum[:, 0:128], dum[:, :],
                                     start=True, stop=True,
                                     skip_group_check=True)

            warm(PRE_WARM)

            # ------- leading DMAs: small consts, then first-chunk data -----
            bb = [cpool.tile([128, 2], F32, tag=f"bb_{k}", name=f"bb_{k}")
                  for k in range(2)]
            for k in range(2):
                nc.sync.dma_start(bb[k][:, :], d_bcols[k * 128:(k + 1) * 128, :])
            svcv = cpool.tile([128, 9], F32, tag="svcv")
            nc.sync.dma_start(svcv[:, :], d_svcv[:, :])
            sv = svcv  # cols 0..7; col 8 is cv
            b0t = [bb[k][:, 0:1] for k in range(2)]
            b1t = [bb[k][:, 1:2] for k in range(2)]
            cv = svcv[:, 8:9]

            ar0 = arpool.tile([128, CH], BF16, tag="ar")
            nc.sync.dma_start(ar0[:, :],
                              d_arows[0:1, 0:CH].broadcast_to([128, CH]))
            rr0 = arpool.tile([128, CH], BF16, tag="rr")
            nc.sync.dma_start(rr0[:, :],
                              d_rrows[0:1, 0:CH].broadcast_to([128, CH]))

            w00 = wpool.tile([128, JC * H], BF16, tag="w")
            nc.sync.dma_start(w00[:, :], d_wmain[0:128, :])

            stripa = cpool.tile([JS, 3 * B], BF16, tag="stripa")
            nc.sync.dma_start(stripa[:, :], d_stripa[:, :])
            a0s = stripa[:, 0:B]
            a1s = stripa[:, B:2 * B]
            rs = stripa[:, 2 * B:3 * B]

            wc0 = cpool.tile([128, 2 * H], BF16, tag="wc0")
            nc.sync.dma_start(wc0[:, :], d_wcol0[:, :])
            wst = cpool.tile([JS, H], BF16, tag="wst")
            nc.sync.dma_start(wst[:, :], d_wstrip[:, :])

            # persistent PSUM accumulators out[h, b]
            ot = [psO.tile([128, B], F32, tag=f"ot{h}", name=f"ot{h}")
                  for h in range(NHT)]
            started = [False] * NHT

            def mm(ht, lhsT, rhs, stop=False):
                nc.tensor.matmul(ot[ht][:, :], lhsT, rhs,
                                 start=not started[ht], stop=stop,
                                 skip_group_check=True)
                started[ht] = True

            def main_iter(c, it, ar, rr, w):
                za = zpool.tile([128, CH], BF16, tag="za")
                nc.scalar.activation(za[:, :], ar[:, :], SIG,
                                     bias=b0t[it], scale=1.0)
                zs = zpool.tile([128, CH], BF16, tag="zs")
                nc.scalar.activation(zs[:, :], ar[:, :], SIG,
                                     bias=b0t[it], scale=-1.0)
                zp = zpool.tile([128, CH], BF16, tag="zp")
                nc.scalar.activation(zp[:, :], ar[:, :], SIG,
                                     bias=0.0, scale=b1t[it])
                zd = zpool.tile([128, CH], BF16, tag="zd")
                nc.scalar.activation(zd[:, :], rr[:, :], SIG,
                                     bias=0.0, scale=b1t[it])

                last = (c == NCHUNK - 1) and (it == 1)
                HCH = CH // 2
                JH = JC // 2
                for hf in range(2):
                    sl = slice(hf * HCH, (hf + 1) * HCH)
                    # combine: ta,td,u1,u2,y1,lk,yl on DVE; tb,tc2 on Pool
                    ta = combpool.tile([128, HCH], BF16, tag="ta")
                    nc.vector.tensor_scalar(ta[:, :], za[:, sl],
                                            sv[:, 0:1], sv[:, 4:5],
                                            AL.mult, AL.add)
                    tb = combpool.tile([128, HCH], BF16, tag="tb")
                    nc.gpsimd.tensor_scalar(tb[:, :], zs[:, sl],
                                            sv[:, 1:2], None, AL.mult)
                    tc2 = combpool.tile([128, HCH], BF16, tag="tc2")
                    nc.gpsimd.tensor_scalar(tc2[:, :], zp[:, sl],
                                            sv[:, 2:3], None, AL.mult)
                    td = combpool.tile([128, HCH], BF16, tag="td")
                    nc.vector.tensor_scalar(td[:, :], zd[:, sl],
                                            sv[:, 3:4], None, AL.mult)
                    u1 = combpool.tile([128, HCH], BF16, tag="u1")
                    nc.vector.tensor_add(u1[:, :], ta[:, :], tb[:, :])
                    u2 = combpool.tile([128, HCH], BF16, tag="u2")
                    nc.vector.tensor_add(u2[:, :], tc2[:, :], td[:, :])
                    y1 = combpool.tile([128, HCH], BF16, tag="y1")
                    nc.vector.tensor_add(y1[:, :], u1[:, :], u2[:, :])
                    lk = combpool.tile([128, HCH], BF16, tag="lk")
                    nc.vector.tensor_scalar(lk[:, :], y1[:, :],
                                            LEAKY, None, AL.mult)
                    yl = ylpool.tile([128, HCH], BF16, tag="yl")
                    nc.vector.tensor_tensor(yl[:, :], y1[:, :], lk[:, :],
                                            AL.max)

                    for jw in range(JH):
                        j = hf * JH + jw
                        for ht in range(NHT):
                            mm(ht,
                               w[:, j * H + ht * 128: j * H + (ht + 1) * 128],
                               yl[:, jw * B:(jw + 1) * B],
                               stop=(last and hf == 1 and jw == JH - 1
                                     and ht == NHT - 1))
                    if last:
                        # no fillers inside the final iteration: they would
                        # run ahead of (and delay) the closing real bursts
                        pass
                    elif c == NCHUNK - 1 and it == 0 and hf == 1:
                        # cover PE through the final iteration's sigmoid +
                        # combine latency so its bursts dispatch fully ramped
                        warm(ND_TAIL)
                    else:
                        warm(ND_HALF)

            def strip_path():
                zas = spool.tile([JS, B], F32, tag="zas")
                nc.scalar.activation(zas[:, :], a0s, SIG,
                                     bias=cv[0:JS, :], scale=1.0)
                zss = spool.tile([JS, B], F32, tag="zss")
                nc.scalar.activation(zss[:, :], a0s, SIG,
                                     bias=cv[0:JS, :], scale=-1.0)
                zps = spool.tile([JS, B], F32, tag="zps")
                nc.scalar.activation(zps[:, :], a1s, SIG,
                                     bias=0.0, scale=cv[0:JS, :])
                zds = spool.tile([JS, B], F32, tag="zds")
                nc.scalar.activation(zds[:, :], rs, SIG,
                                     bias=0.0, scale=cv[0:JS, :])
                t1 = spool.tile([JS, B], F32, tag="t1")
                nc.vector.tensor_scalar(t1[:, :], zas[:, :],
                                        sv[0:JS, 0:1], sv[0:JS, 4:5],
                                        AL.mult, AL.add)
                nc.vector.scalar_tensor_tensor(t1[:, :], zss[:, :],
                                               sv[0:JS, 1:2], t1[:, :],
                                               AL.mult, AL.add)
                nc.vector.scalar_tensor_tensor(t1[:, :], zps[:, :],
                                               sv[0:JS, 2:3], t1[:, :],
                                               AL.mult, AL.add)
                nc.vector.scalar_tensor_tensor(t1[:, :], zds[:, :],
                                               sv[0:JS, 3:4], t1[:, :],
                                               AL.mult, AL.add)
                yls = spool.tile([JS, B], BF16, tag="yls")
                nc.vector.scalar_tensor_tensor(yls[:, :], t1[:, :],
                                               LEAKY, t1[:, :],
                                               AL.mult, AL.max)
                for ht in range(NHT):
                    mm(ht, wst[:, ht * 128:(ht + 1) * 128], yls[:, :])

            def j0_path():
                for it in range(2):
                    za0 = spool.tile([128, 1], F32, tag=f"za0_{it}")
                    nc.scalar.activation(za0[:, :], b0t[it], SIG)
                    zp0 = spool.tile([128, 1], F32, tag=f"zp0_{it}")
                    nc.scalar.activation(zp0[:, :], b1t[it], SIG)
                    t0 = spool.tile([128, 1], F32, tag=f"t0_{it}")
                    nc.vector.tensor_scalar(t0[:, :], za0[:, :],
                                            sv[:, 5:6], sv[:, 4:5],
                                            AL.mult, AL.add)
                    nc.vector.scalar_tensor_tensor(t0[:, :], zp0[:, :],
                                                   sv[:, 6:7], t0[:, :],
                                                   AL.mult, AL.add)
                    yl0 = spool.tile([128, 1], F32, tag=f"yl0_{it}")
                    nc.vector.scalar_tensor_tensor(yl0[:, :], t0[:, :],
                                                   LEAKY, t0[:, :],
                                                   AL.mult, AL.max)
                    yj0 = spool.tile([128, B], BF16, tag=f"yj0_{it}")
                    nc.vector.tensor_copy(yj0[:, :],
                                          yl0[:, 0:1].broadcast_to([128, B]))
                    for ht in range(NHT):
                        mm(ht,
                           wc0[:, it * H + ht * 128: it * H + (ht + 1) * 128],
                           yj0[:, :])

            # ---------------- main schedule ----------------
            for c in range(NCHUNK):
                if c == 0:
                    ar, rr = ar0, rr0
                else:
                    ar = arpool.tile([128, CH], BF16, tag="ar")
                    nc.sync.dma_start(
                        ar[:, :],
                        d_arows[0:1, c * CH:(c + 1) * CH]
                        .broadcast_to([128, CH]))
                    rr = arpool.tile([128, CH], BF16, tag="rr")
                    nc.sync.dma_start(
                        rr[:, :],
                        d_rrows[0:1, c * CH:(c + 1) * CH]
                        .broadcast_to([128, CH]))

                for it in range(2):
                    if c == 0 and it == 0:
                        w = w00
                    else:
                        w = wpool.tile([128, JC * H], BF16, tag="w")
                        r0 = (it * NCHUNK + c) * 128
                        nc.sync.dma_start(w[:, :], d_wmain[r0:r0 + 128, :])
                    main_iter(c, it, ar, rr, w)

                if c == 0:
                    strip_path()
                    j0_path()

            # ---------------- write out partial y2 [128, 4*B] --------------
            ob = finpool.tile([128, NHT * B], F32, tag="ob")
            for ht in range(NHT):
                osl = ob[:, ht * B:(ht + 1) * B]
                if ht % 2 == 0:
                    nc.scalar.copy(osl, ot[ht][:, :])
                else:
                    nc.vector.tensor_copy(osl, ot[ht][:, :])
            nc.sync.dma_start(d_out[:, :], ob[:, :])

    nc.finalize()
    return nc


_CACHED_NC = None


def _get_program():
    global _CACHED_NC
    if _CACHED_NC is None:
        _CACHED_NC = build_program()
    return _CACHED_NC


def make_in_maps(x1, x3, conv_w, conv_b, bn_gamma, bn_beta, bn_mean, bn_var,
                 fc_w, fc_b, out_w, out_b):
    x1 = np.asarray(x1, np.float32)
    x3 = np.asarray(x3, np.float32)
    fc_w = np.asarray(fc_w, np.float32)

    g = float(np.asarray(bn_gamma).reshape(-1)[0]) / float(
        np.sqrt(np.asarray(bn_var).reshape(-1)[0] + BN_EPS))
    s = np.asarray(conv_w, np.float32).reshape(-1) * g
    off = (float(np.asarray(conv_b).reshape(-1)[0])
           - float(np.asarray(bn_mean).reshape(-1)[0])) * g \
        + float(np.asarray(bn_beta).reshape(-1)[0])

    svcv = np.zeros((128, 9), np.float32)
    svcv[:, 0], svcv[:, 1], svcv[:, 2], svcv[:, 3] = s[0], s[1], s[2], s[3]
    svcv[:, 4] = off
    svcv[:, 5] = s[0] + s[1]
    svcv[:, 6] = s[2] + s[3]
    svcv[:, 8] = x3[-1]

    b0 = np.concatenate([[0.0], x3]).astype(np.float32)  # (257,)
    b1 = np.concatenate([[1.0], x3]).astype(np.float32)
    bcols = np.stack([b0[:256], b1[:256]], axis=1).astype(np.float32)  # (256,2)

    # fc_w (H, 66049) with k = i*257+j  ->  W3 [i, j, h]
    w3 = np.ascontiguousarray(fc_w.reshape(H, NP, NP).transpose(1, 2, 0))

    x1T = np.ascontiguousarray(x1.T)                     # (256 j-1, 256 b)
    rT = (1.0 / (x1T + np.float32(EPS))).astype(np.float32)

    bf = np.dtype("bfloat16")
    in_maps = []
    for core in range(NCORE):
        jsl = slice(1 + JPC * core, 1 + JPC * (core + 1))  # j values
        xs = x1T[JPC * core: JPC * (core + 1), :]          # (32, 256)
        rsl = rT[JPC * core: JPC * (core + 1), :]

        arows = np.ascontiguousarray(xs.reshape(1, JPC * B)).astype(bf)
        rrows = np.ascontiguousarray(rsl.reshape(1, JPC * B)).astype(bf)

        # wmain [it, chunk, i(128), jw(8), h] -> [(2*4)*128, 8*512]
        wm = np.empty((2, NCHUNK, 128, JC, H), np.float32)
        for it in range(2):
            for ch in range(NCHUNK):
                j0 = 1 + JPC * core + ch * JC
                wm[it, ch] = w3[it * 128:(it + 1) * 128, j0:j0 + JC, :]
        wmain = np.ascontiguousarray(
            wm.reshape(2 * NCHUNK * 128, JC * H)).astype(bf)

        # wcol0 [128 i, 2 it * H]
        wcol0 = np.concatenate(
            [w3[0:128, 0, :] / 8.0, w3[128:256, 0, :] / 8.0],
            axis=1).astype(bf)

        wstrip = np.empty((JPC + 1, H), np.float32)
        wstrip[0] = w3[256, 0, :] / 8.0
        wstrip[1:] = w3[256, jsl, :]
        wstrip = wstrip.astype(bf)

        stripa = np.empty((JPC + 1, 3 * B), np.float32)
        stripa[0, 0:B] = 0.0                # a0 at j=0
        stripa[1:, 0:B] = xs
        stripa[0, B:2 * B] = 1.0            # a1 at j=0
        stripa[1:, B:2 * B] = xs
        stripa[0, 2 * B:] = 1.0 / (1.0 + EPS)
        stripa[1:, 2 * B:] = rsl

        in_maps.append({
            "arows": arows, "rrows": rrows, "bcols": bcols, "svcv": svcv,
            "wmain": wmain, "wcol0": wcol0, "wstrip": wstrip,
            "stripa": stripa.astype(bf),
        })
    return in_maps


def kernel(**inputs):
    in_maps = make_in_maps(**inputs)
    nc = _get_program()
    res = run_bass_kernel_spmd(nc, in_maps, list(range(NCORE)))

    acc = np.zeros((128, NHT * B), np.float32)
    for core in range(NCORE):
        acc += np.asarray(res.results[core]["out"], np.float32)
    # acc[p, ht*B + b] = y2[ht*128 + p, b]
    y2 = acc.reshape(128, NHT, B).transpose(1, 0, 2).reshape(H, B)
    y2 = y2.T + np.asarray(inputs["fc_b"], np.float32).reshape(1, H)
    y2 = np.maximum(y2, 0.0)
    logits = y2 @ np.asarray(inputs["out_w"], np.float32).T \
        + np.asarray(inputs["out_b"], np.float32).reshape(1, C)
    return logits.astype(np.float32)
